# revision 56
# baseline (speedup 1.0000x reference)
"""MoE gating kernel for Trainium2 (Bass/Tile), data-parallel over 8 NeuronCores.

Computes: logits = x @ W_g.T ; top-2 values; softmax over the 2 values.
  p1 = sigmoid(v1 - v2), p2 = sigmoid(v2 - v1)  (v1 >= v2 the top-2 logits)

Sharding: tokens split 8 ways (2048 tokens/core), W_g replicated.

Per-core structure: all 16 x-tiles [128, 2048] are preloaded into SBUF (DMA
streams at full rate with no WAR stalls), then per 256-token group:
  - PE-transpose x into xT [128 d, 16k * 256 t] via one-bank PSUM staging
  - 16 accumulating matmuls (float32r, N=256): logitsT [64 e, 256 t]
  - drain + PE-transpose back to [128 t, 64 e], DVE Max8 top-2, ACT sigmoid
float32r (TF32-like) matmul runs 4x faster than fp32 on the PE; it adds
~2.3e-4 max abs error on the output probabilities vs the fp32 reference.
Set mm_f32r=False in _build/_run for bit-accurate fp32 (~1.5x slower).
"""

import sys

sys.path.insert(0, "/opt/trn_rl_repo")

from contextlib import ExitStack

import numpy as np

import concourse.bass as bass
import concourse.bacc as bacc
import concourse.mybir as mybir
from concourse import masks
from concourse.tile import TileContext
from concourse.bass_utils import run_bass_kernel_spmd

TOKENS = 16384
DIM = 2048
E = 64  # num experts
NCORES = 8
TPC = TOKENS // NCORES  # tokens per core
P = 128
KT = DIM // P  # 16 contraction tiles
G = 256  # token group (moving-dim of the big matmul)
NG = TPC // G  # 4 groups per core
TB = G // P  # 4 token blocks per group

F32 = mybir.dt.float32
F32R = mybir.dt.float32r

# which k-drains go to the scalar engine (ACT) instead of DVE (load balance)
ACT_DRAIN_EVERY = 8  # k % 8 == 7 -> ACT


def _emit(tc: TileContext, ctx: ExitStack, x_ap, wg_ap, out_ap, mm_f32r=True):
    nc = tc.nc

    singles = ctx.enter_context(tc.tile_pool(name="singles", bufs=1))
    xpool = ctx.enter_context(tc.tile_pool(name="xpool", bufs=1))
    xtpool = ctx.enter_context(tc.tile_pool(name="xtpool", bufs=2))
    ltpool = ctx.enter_context(tc.tile_pool(name="ltpool", bufs=2))
    spool = ctx.enter_context(tc.tile_pool(name="spool", bufs=4))
    opool = ctx.enter_context(tc.tile_pool(name="opool", bufs=4))
    psum_t = ctx.enter_context(tc.tile_pool(name="psum_t", bufs=3, space="PSUM"))
    psum_l = ctx.enter_context(tc.tile_pool(name="psum_l", bufs=2, space="PSUM"))
    psum_f = ctx.enter_context(tc.tile_pool(name="psum_f", bufs=3, space="PSUM"))

    ident = singles.tile([P, P], F32)
    masks.make_identity(nc, ident[:])

    # --- one-time: load W_g [64, 2048], transpose to wgT [128 d, 16k * 64 e] ---
    mmdt = F32R if mm_f32r else F32

    # W_g loaded as a flat [128, 1024] reshape: row (2e+h) = W_g[e, 1024h:+1024].
    # Full 128 partitions -> fast DMA; 8 PE transposes instead of 16.
    # (SWDGE queue: runs concurrently with the x stream on the sync queue)
    wg_sb = singles.tile([P, DIM // 2], F32)
    nc.gpsimd.dma_start(out=wg_sb[:], in_=wg_ap.rearrange("e (h c) -> (e h) c", h=2))

    # preload ALL of this core's x into SBUF (16 tiles x 8KB/partition);
    # DMA then runs start-to-finish at full rate with no WAR stalls.
    # Group 0's four tiles are loaded first, k-slice-interleaved, so the
    # k-major transpose+drain pipeline can start after ~256KB instead of 4MB.
    all_x = []
    for t in range(NG * TB):
        xt_in = xpool.tile([P, DIM], F32, tag=f"x{t}")
        all_x.append(xt_in)
    # Only the first k-wave is split out (small 2KB-row descriptors run at ~1/3
    # DMA rate — keep that to the minimum needed to start the PE early).
    KCH = 4  # k-tiles in the first-wave chunk
    for tb in range(TB):
        nc.sync.dma_start(
            out=all_x[tb][:, : KCH * P],
            in_=x_ap[tb * P : (tb + 1) * P, : KCH * P],
        )
    for tb in range(TB):
        nc.sync.dma_start(
            out=all_x[tb][:, KCH * P :],
            in_=x_ap[tb * P : (tb + 1) * P, KCH * P :],
        )
    for t in range(TB, NG * TB):
        nc.sync.dma_start(out=all_x[t][:], in_=x_ap[t * P : (t + 1) * P, :])
    # wgT[c, j, 2e+h] = W_g[e, 1024h + 128j + c]; see wgT_k for k-tile APs.
    # Built after group 0's x-transposes (PE FIFO order): the slow SWDGE W_g
    # load then overlaps group 0's transpose work instead of gating PE start.
    wgT = singles.tile([P, KT // 2, P], mmdt)

    def build_wgT():
        for j in range(KT // 2):
            pt = psum_f.tile([P, P], F32, tag="fin_ps")
            nc.tensor.matmul(
                pt[:],
                wg_sb[:, j * P : (j + 1) * P],
                ident[:],
                is_transpose=True,
            )
            nc.vector.tensor_copy(wgT[:, j, :], pt[:])

    def wgT_k(k):
        # [128 d-part, 64 experts] for k-tile k: d = 1024h + 128j + c
        h, j = divmod(k, KT // 2)
        base = wgT[:, j, :]
        return bass.AP(
            tensor=base.tensor,
            offset=base.offset + h,
            ap=[base.ap[0], [2, E]],
        )

    def epilogue(g, lp):
        # back to token-major + top-2 + softmax (runs one group late)
        lt = ltpool.tile([E, G], F32)
        for tb in range(TB):
            nc.vector.tensor_copy(
                lt[:, tb * P : (tb + 1) * P], lp[:, tb * P : (tb + 1) * P]
            )
            fp = psum_f.tile([P, E], F32, tag="fin_ps")
            nc.tensor.matmul(
                fp[:],
                lt[:, tb * P : (tb + 1) * P],
                ident[:E, :E],
                is_transpose=True,
            )
            max8 = spool.tile([P, 8], F32)
            nc.vector.max(out=max8[:], in_=fp[:])
            d21 = spool.tile([P, 1], F32)
            nc.gpsimd.tensor_sub(d21[:], max8[:, 1:2], max8[:, 0:1])  # v2 - v1
            ot = opool.tile([P, 2], F32)
            nc.scalar.activation(
                ot[:, 0:1], d21[:], mybir.ActivationFunctionType.Sigmoid, scale=-1.0
            )
            nc.scalar.activation(
                ot[:, 1:2], d21[:], mybir.ActivationFunctionType.Sigmoid, scale=1.0
            )
            r0 = g * G + tb * P
            nc.sync.dma_start(out=out_ap[r0 : r0 + P, :], in_=ot[:])

    pending = None  # (g, lp) awaiting epilogue
    for g in range(NG):
        xtiles = all_x[g * TB : (g + 1) * TB]

        # transpose into xT [128 d, k * 512 t]
        xt = xtpool.tile([P, KT * G], mmdt)
        for k in range(KT):
            pt = psum_t.tile([P, G], F32)
            for tb in range(TB):
                nc.tensor.matmul(
                    pt[:, tb * P : (tb + 1) * P],
                    xtiles[tb][:, k * P : (k + 1) * P],
                    ident[:],
                    is_transpose=True,
                )
            dst = xt[:, k * G : (k + 1) * G]
            if k % ACT_DRAIN_EVERY == ACT_DRAIN_EVERY - 1:
                nc.scalar.copy(dst, pt[:])
            else:
                nc.vector.tensor_copy(dst, pt[:])

        if g == 0:
            build_wgT()

        # logitsT [64 e, 256 t] = sum_k wgT_k.T @ xT_k
        lp = psum_l.tile([E, G], F32)
        for k in range(KT):
            nc.tensor.matmul(
                lp[:],
                wgT_k(k),
                xt[:, k * G : (k + 1) * G],
                start=(k == 0),
                stop=(k == KT - 1),
            )

        if pending is not None:
            epilogue(*pending)
        pending = (g, lp)
    epilogue(*pending)


_NC_CACHE = {}


def _build(mm_f32r=True):
    key = ("nc", mm_f32r)
    if key in _NC_CACHE:
        return _NC_CACHE[key]
    nc = bacc.Bacc(trn_type="TRN2")
    x = nc.dram_tensor("x", [TPC, DIM], F32, kind="ExternalInput")
    wg = nc.dram_tensor("w_g", [E, DIM], F32, kind="ExternalInput")
    out = nc.dram_tensor("out", [TPC, 2], F32, kind="ExternalOutput")
    with TileContext(nc) as tc, ExitStack() as ctx:
        _emit(tc, ctx, x.ap(), wg.ap(), out.ap(), mm_f32r=mm_f32r)
    if not nc.is_finalized():
        nc.finalize()
    _NC_CACHE[key] = nc
    return nc


def _run(x, W_g, trace=False, mm_f32r=True):
    nc = _build(mm_f32r=mm_f32r)
    x = np.ascontiguousarray(np.asarray(x, dtype=np.float32))
    W_g = np.ascontiguousarray(np.asarray(W_g, dtype=np.float32))
    in_maps = [
        {"x": np.ascontiguousarray(x[c * TPC : (c + 1) * TPC]), "w_g": W_g}
        for c in range(NCORES)
    ]
    res = run_bass_kernel_spmd(nc, in_maps, core_ids=list(range(NCORES)), trace=trace)
    out = np.concatenate([r["out"] for r in res.results], axis=0)
    return out, res


def kernel(x, W_g):
    out, _ = _run(x, W_g, trace=False)
    return out


def kernel_profiled(x, W_g, mm_f32r=True):
    out, res = _run(x, W_g, trace=True, mm_f32r=mm_f32r)
    return out, res


# revision 57
# speedup vs baseline: 1.0962x; 1.0962x over previous
"""MoE gating kernel for Trainium2 (Bass/Tile), data-parallel over 8 NeuronCores.

Computes: logits = x @ W_g.T ; top-2 values; softmax over the 2 values.
  p1 = sigmoid(v1 - v2), p2 = sigmoid(v2 - v1)  (v1 >= v2 the top-2 logits)

Sharding: tokens split 8 ways (2048 tokens/core), W_g replicated.

Per-core structure: all 16 x-tiles [128, 2048] are preloaded into SBUF (DMA
streams at full rate with no WAR stalls), then per 256-token group:
  - PE-transpose x into xT [128 d, 16k * 256 t] via one-bank PSUM staging
  - 16 accumulating matmuls (float32r, N=256): logitsT [64 e, 256 t]
  - drain + PE-transpose back to [128 t, 64 e], DVE Max8 top-2, ACT sigmoid
float32r (TF32-like) matmul runs 4x faster than fp32 on the PE; it adds
~2.3e-4 max abs error on the output probabilities vs the fp32 reference.
Set mm_f32r=False in _build/_run for bit-accurate fp32 (~1.5x slower).
"""

import sys

sys.path.insert(0, "/opt/trn_rl_repo")

from contextlib import ExitStack

import numpy as np

import concourse.bass as bass
import concourse.bacc as bacc
import concourse.mybir as mybir
from concourse import masks
from concourse.tile import TileContext
from concourse.bass_utils import run_bass_kernel_spmd

TOKENS = 16384
DIM = 2048
E = 64  # num experts
NCORES = 8
TPC = TOKENS // NCORES  # tokens per core
P = 128
KT = DIM // P  # 16 contraction tiles
G = 256  # token group (moving-dim of the big matmul)
NG = TPC // G  # 4 groups per core
TB = G // P  # 4 token blocks per group

F32 = mybir.dt.float32
F32R = mybir.dt.float32r

# which k-drains go to the scalar engine (ACT) instead of DVE (load balance)
ACT_DRAIN_EVERY = 8  # k % 8 == 7 -> ACT


def _emit(tc: TileContext, ctx: ExitStack, x_ap, wg_ap, out_ap, mm_f32r=True):
    nc = tc.nc

    singles = ctx.enter_context(tc.tile_pool(name="singles", bufs=1))
    xpool = ctx.enter_context(tc.tile_pool(name="xpool", bufs=1))
    xtpool = ctx.enter_context(tc.tile_pool(name="xtpool", bufs=2))
    ltpool = ctx.enter_context(tc.tile_pool(name="ltpool", bufs=2))
    spool = ctx.enter_context(tc.tile_pool(name="spool", bufs=4))
    opool = ctx.enter_context(tc.tile_pool(name="opool", bufs=4))
    psum_t = ctx.enter_context(tc.tile_pool(name="psum_t", bufs=3, space="PSUM"))
    psum_l = ctx.enter_context(tc.tile_pool(name="psum_l", bufs=2, space="PSUM"))
    psum_f = ctx.enter_context(tc.tile_pool(name="psum_f", bufs=3, space="PSUM"))

    ident = singles.tile([P, P], F32)
    masks.make_identity(nc, ident[:])

    # --- one-time: load W_g [64, 2048], transpose to wgT [128 d, 16k * 64 e] ---
    mmdt = F32R if mm_f32r else F32

    # W_g loaded as a flat [128, 1024] reshape: row (2e+h) = W_g[e, 1024h:+1024].
    # Full 128 partitions -> fast DMA; 8 PE transposes instead of 16.
    # (SWDGE queue: runs concurrently with the x stream on the sync queue)
    wg_sb = singles.tile([P, DIM // 2], F32)
    nc.gpsimd.dma_start(out=wg_sb[:], in_=wg_ap.rearrange("e (h c) -> (e h) c", h=2))

    # preload ALL of this core's x into SBUF (16 tiles x 8KB/partition);
    # DMA then runs start-to-finish at full rate with no WAR stalls.
    # Group 0's four tiles are loaded first, k-slice-interleaved, so the
    # k-major transpose+drain pipeline can start after ~256KB instead of 4MB.
    all_x = []
    for t in range(NG * TB):
        xt_in = xpool.tile([P, DIM], F32, tag=f"x{t}")
        all_x.append(xt_in)
    KCH = 4  # k-tiles per first-group load chunk
    for kc in range(0, KT, KCH):
        for tb in range(TB):
            nc.sync.dma_start(
                out=all_x[tb][:, kc * P : (kc + KCH) * P],
                in_=x_ap[tb * P : (tb + 1) * P, kc * P : (kc + KCH) * P],
            )
    for t in range(TB, NG * TB):
        nc.sync.dma_start(out=all_x[t][:], in_=x_ap[t * P : (t + 1) * P, :])
    # wgT[c, j, 2e+h] = W_g[e, 1024h + 128j + c]; see wgT_k for k-tile APs.
    # Built after group 0's x-transposes (PE FIFO order): the slow SWDGE W_g
    # load then overlaps group 0's transpose work instead of gating PE start.
    wgT = singles.tile([P, KT // 2, P], mmdt)

    def build_wgT():
        for j in range(KT // 2):
            pt = psum_f.tile([P, P], F32, tag="fin_ps")
            nc.tensor.matmul(
                pt[:],
                wg_sb[:, j * P : (j + 1) * P],
                ident[:],
                is_transpose=True,
            )
            nc.vector.tensor_copy(wgT[:, j, :], pt[:])

    def wgT_k(k):
        # [128 d-part, 64 experts] for k-tile k: d = 1024h + 128j + c
        h, j = divmod(k, KT // 2)
        base = wgT[:, j, :]
        return bass.AP(
            tensor=base.tensor,
            offset=base.offset + h,
            ap=[base.ap[0], [2, E]],
        )

    def epilogue(g, lp):
        # back to token-major + top-2 + softmax (runs one group late)
        lt = ltpool.tile([E, G], F32)
        for tb in range(TB):
            nc.vector.tensor_copy(
                lt[:, tb * P : (tb + 1) * P], lp[:, tb * P : (tb + 1) * P]
            )
            fp = psum_f.tile([P, E], F32, tag="fin_ps")
            nc.tensor.matmul(
                fp[:],
                lt[:, tb * P : (tb + 1) * P],
                ident[:E, :E],
                is_transpose=True,
            )
            max8 = spool.tile([P, 8], F32)
            nc.vector.max(out=max8[:], in_=fp[:])
            d21 = spool.tile([P, 1], F32)
            nc.gpsimd.tensor_sub(d21[:], max8[:, 1:2], max8[:, 0:1])  # v2 - v1
            ot = opool.tile([P, 2], F32)
            nc.scalar.activation(
                ot[:, 0:1], d21[:], mybir.ActivationFunctionType.Sigmoid, scale=-1.0
            )
            nc.scalar.activation(
                ot[:, 1:2], d21[:], mybir.ActivationFunctionType.Sigmoid, scale=1.0
            )
            r0 = g * G + tb * P
            nc.sync.dma_start(out=out_ap[r0 : r0 + P, :], in_=ot[:])

    pending = None  # (g, lp) awaiting epilogue
    for g in range(NG):
        xtiles = all_x[g * TB : (g + 1) * TB]

        # transpose into xT [128 d, k * 512 t]
        xt = xtpool.tile([P, KT * G], mmdt)
        for k in range(KT):
            pt = psum_t.tile([P, G], F32)
            for tb in range(TB):
                nc.tensor.matmul(
                    pt[:, tb * P : (tb + 1) * P],
                    xtiles[tb][:, k * P : (k + 1) * P],
                    ident[:],
                    is_transpose=True,
                )
            dst = xt[:, k * G : (k + 1) * G]
            if k % ACT_DRAIN_EVERY == ACT_DRAIN_EVERY - 1:
                nc.scalar.copy(dst, pt[:])
            else:
                nc.vector.tensor_copy(dst, pt[:])

        if g == 0:
            build_wgT()

        # logitsT [64 e, 256 t] = sum_k wgT_k.T @ xT_k
        lp = psum_l.tile([E, G], F32)
        for k in range(KT):
            nc.tensor.matmul(
                lp[:],
                wgT_k(k),
                xt[:, k * G : (k + 1) * G],
                start=(k == 0),
                stop=(k == KT - 1),
            )

        if pending is not None:
            epilogue(*pending)
        pending = (g, lp)
    epilogue(*pending)


_NC_CACHE = {}


def _build(mm_f32r=True):
    key = ("nc", mm_f32r)
    if key in _NC_CACHE:
        return _NC_CACHE[key]
    nc = bacc.Bacc(trn_type="TRN2")
    x = nc.dram_tensor("x", [TPC, DIM], F32, kind="ExternalInput")
    wg = nc.dram_tensor("w_g", [E, DIM], F32, kind="ExternalInput")
    out = nc.dram_tensor("out", [TPC, 2], F32, kind="ExternalOutput")
    with TileContext(nc) as tc, ExitStack() as ctx:
        _emit(tc, ctx, x.ap(), wg.ap(), out.ap(), mm_f32r=mm_f32r)
    if not nc.is_finalized():
        nc.finalize()
    _NC_CACHE[key] = nc
    return nc


def _run(x, W_g, trace=False, mm_f32r=True):
    nc = _build(mm_f32r=mm_f32r)
    x = np.ascontiguousarray(np.asarray(x, dtype=np.float32))
    W_g = np.ascontiguousarray(np.asarray(W_g, dtype=np.float32))
    in_maps = [
        {"x": np.ascontiguousarray(x[c * TPC : (c + 1) * TPC]), "w_g": W_g}
        for c in range(NCORES)
    ]
    res = run_bass_kernel_spmd(nc, in_maps, core_ids=list(range(NCORES)), trace=trace)
    out = np.concatenate([r["out"] for r in res.results], axis=0)
    return out, res


def kernel(x, W_g):
    out, _ = _run(x, W_g, trace=False)
    return out


def kernel_profiled(x, W_g, mm_f32r=True):
    out, res = _run(x, W_g, trace=True, mm_f32r=mm_f32r)
    return out, res
